# revision 1
# baseline (speedup 1.0000x reference)
"""Trainium2 Bass kernel for nn_MixtureOfAdaptors (moe_routing).

The reference routing collapses to expert 0 with weight 1.0, so the module is
exactly: out = x @ W[0].T + b[0], with x [65536, 1024] fp32.

Strategy (8 NeuronCores, data-parallel over tokens):
  - Host: shard x by token into 8 x [8192, 1024]; transpose each shard to
    feature-major [1024, 8192] (the PE contracts over the partition axis, so
    both matmul operands need the hidden dim on partitions); round x and W[0].T
    to the fp32r format (fp32 with 11 explicit mantissa bits, RNE) so the
    TensorE can run fp32r matmuls at 1 column/cycle (4x faster than fp32).
  - Device (per core): keep W[0].T resident in SBUF as fp32r (8 tiles of
    [128, 1024], one per 128-wide hidden block); stream 1024-token chunks of
    x.T as 8 per-block DMAs (4KB contiguous runs, fine-grained DMA->matmul
    dependencies); 8 accumulating fp32r matmuls per (128-token, 512-feature)
    PSUM tile, all 8 PSUM banks in flight; bias-add on VectorE during
    PSUM->SBUF copyback; DMA out in natural token-major layout.

    Measured steady-state: ~255-260us per core (PE-bound; 1024 matmul
    instructions x ~250ns; DMA ~237us overlapped).
"""

import sys

if "/opt/trn_rl_repo" not in sys.path:
    sys.path.insert(0, "/opt/trn_rl_repo")

from contextlib import ExitStack

import numpy as np

import concourse.bass as bass
import concourse.tile as tile
from concourse import bacc, mybir
from concourse.bass_utils import run_bass_kernel_spmd

dt = mybir.dt

BATCH = 65536
HIDDEN = 1024
NCORES = 8
SHARD = BATCH // NCORES  # 8192 tokens per core
KD = HIDDEN // 128  # 8 hidden-dim blocks of 128
CHUNK = 1024  # tokens per streamed x chunk (4KB contiguous DMA runs)
NCHUNKS = SHARD // CHUNK
SM = CHUNK // 128


def round_fp32r(a: np.ndarray) -> np.ndarray:
    """Round fp32 to fp32r: 11 explicit mantissa bits, round-to-nearest-even."""
    bits = a.view(np.uint32).astype(np.uint64)
    lsb = (bits >> 12) & 1
    rounded = (bits + 0x7FF + lsb) & ~np.uint64(0xFFF)
    return rounded.astype(np.uint32).view(np.float32)


def build_program(loop_reps: int = 0, bench_mode: bool = False):
    """Build the per-core Bass program. loop_reps>0 wraps the main loop in a
    hardware For_i that repeats the whole computation (for benchmarking).

    bench_mode=True keeps x and out in Internal DRAM (no host transfer) so
    wall-clock timing of repeated runs is dominated by device execution; a tiny
    external output preserves a data dependency on the computation."""
    nc = bacc.Bacc("TRN2", debug=False, enable_asserts=True, num_devices=NCORES)
    io_kind = "Internal" if bench_mode else None
    xT_d = nc.dram_tensor(
        "xT", [HIDDEN, SHARD], dt.float32r, kind=io_kind or "ExternalInput"
    ).ap()
    w_d = nc.dram_tensor("w0t", [HIDDEN, HIDDEN], dt.float32r, kind="ExternalInput").ap()
    b_d = nc.dram_tensor("b0", [1, HIDDEN], dt.float32, kind="ExternalInput").ap()
    out_d = nc.dram_tensor(
        "out", [SHARD, HIDDEN], dt.float32, kind=io_kind or "ExternalOutput"
    ).ap()
    done_d = (
        nc.dram_tensor("done", [1, 16], dt.float32, kind="ExternalOutput").ap()
        if bench_mode
        else None
    )

    xT_v = xT_d.rearrange("(kd p) n -> p kd n", p=128)  # [128, 8, 8192]
    w_v = w_d.rearrange("(kd p) o -> p kd o", p=128)  # [128, 8, 1024]

    with tile.TileContext(nc) as tc:
        with ExitStack() as ctx:
            singles = ctx.enter_context(tc.tile_pool(name="singles", bufs=1))
            xpool = ctx.enter_context(tc.tile_pool(name="xpool", bufs=4))
            opool = ctx.enter_context(tc.tile_pool(name="opool", bufs=4))
            pspool = ctx.enter_context(tc.tile_pool(name="pspool", bufs=8, space="PSUM"))

            # Resident W[0].T in fp32r (one tile per 128-wide hidden block so
            # matmuls depend only on the slice they read) and broadcast bias.
            wts = []
            for kd in range(KD):
                wk = singles.tile([128, HIDDEN], dt.float32r, name=f"wt{kd}")
                nc.sync.dma_start(wk, w_v[:, kd, :])
                wts.append(wk)
            bias = singles.tile([128, HIDDEN], dt.float32, name="bias")
            nc.gpsimd.dma_start(
                bias, bass.AP(b_d.tensor, 0, [[0, 128], [1, HIDDEN]])
            )

            def chunk_body(ch: int):
                # one DMA + one tile per 128-wide hidden block: kd-block k's
                # matmuls unblock as soon as its slice lands
                xks = []
                for kd in range(KD):
                    xk = xpool.tile([128, CHUNK], dt.float32r, name=f"xk{kd}", tag=f"xk{kd}")
                    nc.sync.dma_start(xk, xT_v[:, kd, ch * CHUNK : (ch + 1) * CHUNK])
                    xks.append(xk)
                for sm in range(SM):
                    tok = ch * CHUNK + sm * 128
                    osb = opool.tile([128, HIDDEN], dt.float32, name="osb", tag="osb")
                    ps0 = pspool.tile([128, 512], dt.float32, name="ps0", tag="ps")
                    ps1 = pspool.tile([128, 512], dt.float32, name="ps1", tag="ps")
                    for kd in range(KD):
                        lhsT = xks[kd][:, sm * 128 : (sm + 1) * 128]
                        nc.tensor.matmul(
                            ps0, lhsT, wts[kd][:, 0:512],
                            start=(kd == 0), stop=(kd == KD - 1),
                        )
                        nc.tensor.matmul(
                            ps1, lhsT, wts[kd][:, 512:1024],
                            start=(kd == 0), stop=(kd == KD - 1),
                        )
                    nc.vector.tensor_add(osb[:, 0:512], ps0, bias[:, 0:512])
                    nc.vector.tensor_add(osb[:, 512:1024], ps1, bias[:, 512:1024])
                    nc.sync.dma_start(out_d[tok : tok + 128, :], osb)

            if bench_mode:
                # fp32r tiles may contain arbitrary bits in bench mode (x is
                # uninitialized Internal DRAM); zero the x region once so the
                # PE never chews on NaN/Inf patterns.
                zro = singles.tile([128, KD, 256], dt.float32r, name="zro")
                nc.vector.memset(zro.bitcast(dt.float32), 0.0)
                for zc in range(SHARD // 256):
                    nc.sync.dma_start(xT_v[:, :, zc * 256 : (zc + 1) * 256], zro)

            if loop_reps > 0:
                with tc.For_i(0, loop_reps, 1):
                    for ch in range(NCHUNKS):
                        chunk_body(ch)
            else:
                for ch in range(NCHUNKS):
                    chunk_body(ch)

            if done_d is not None:
                dsb = singles.tile([1, 16], dt.float32, name="dsb")
                nc.vector.tensor_copy(dsb, bias[0:1, 0:16])
                nc.sync.dma_start(done_d, dsb)

    nc.compile()
    return nc


_nc_cache: dict[tuple, object] = {}


def _get_nc(loop_reps: int = 0, bench_mode: bool = False):
    key = (loop_reps, bench_mode)
    if key not in _nc_cache:
        _nc_cache[key] = build_program(loop_reps, bench_mode)
    return _nc_cache[key]


def prepare_in_maps(x: np.ndarray, W: np.ndarray, b: np.ndarray):
    w0t_r = round_fp32r(np.ascontiguousarray(W[0].T))
    b0 = np.ascontiguousarray(b[0].reshape(1, HIDDEN)).astype(np.float32)
    in_maps = []
    for c in range(NCORES):
        x_c = x[c * SHARD : (c + 1) * SHARD]
        xT_c = round_fp32r(np.ascontiguousarray(x_c.T))
        in_maps.append({"xT": xT_c, "w0t": w0t_r, "b0": b0})
    return in_maps


def kernel(x, routing_vectors, W, b):
    x = np.asarray(x, dtype=np.float32)
    W = np.asarray(W, dtype=np.float32)
    b = np.asarray(b, dtype=np.float32)
    nc = _get_nc(0)
    in_maps = prepare_in_maps(x, W, b)
    res = run_bass_kernel_spmd(nc, in_maps, core_ids=list(range(NCORES)))
    return np.concatenate([res.results[c]["out"] for c in range(NCORES)], axis=0)



# revision 3
# speedup vs baseline: 1.3436x; 1.3436x over previous
"""Trainium2 Bass kernel for nn_MixtureOfAdaptors (moe_routing).

out = x @ W[0].T + b[0] (routing collapses to expert 0), x [65536, 1024] fp32.

bf16 single-pass matmul (rel err ~2e-3 vs the 2e-2 gate), data-parallel over
8 cores (8192 tokens each). Why bf16: fp8 DoubleRow measures only ~2x bf16
per output element on TRN2 (cost model's 4x is wrong), and every fp8
operand-splitting scheme that passes the accuracy gate needs >= 2.25 passes,
i.e. slower than one bf16 pass; bf16 also sustains a higher PE clock than
fp32r (~2.2-2.3 GHz vs ~2.0, power-dependent), which is where the win over
the fp32r baseline comes from.

Device layout: out.T tiles [128 features, tokens] so the per-feature bias
rides the partition axis; the PSUM drain fuses bias-add + fp16 convert in a
single ACT or DVE pass (alternating engines). Stationary = W tiles
[128k, 128feat], 4 moving [128k, 512tok] matmuls per load (LDWEIGHTS is
measured-free when matmuls are back-to-back); PSUM as two ping-ponging
4-bank groups; token-major loop so x-tile DMAs for the next For_i iteration
land while compute proceeds. Host does layout prep only: bf16 rounding,
transposes, final fp16->fp32 upcast.

Measured steady-state: ~230us/pass (best-case ~228; device shows +-10-17%
run-to-run noise) vs 262us for the fp32r baseline.
"""

import sys

if "/opt/trn_rl_repo" not in sys.path:
    sys.path.insert(0, "/opt/trn_rl_repo")

from contextlib import ExitStack

import ml_dtypes
import numpy as np

import concourse.bass as bass
import concourse.tile as tile
from concourse import bacc, mybir
from concourse.bass_utils import run_bass_kernel_spmd

dt = mybir.dt
BF16 = ml_dtypes.bfloat16

BATCH = 65536
HIDDEN = 1024
NCORES = 8
SHARD = BATCH // NCORES  # 8192 tokens per core
NKD = HIDDEN // 128  # 8 contraction blocks
NOB = HIDDEN // 128  # 8 output-feature blocks
GTOK = 2048  # tokens per PSUM group (4 banks of [128, 512] fp32)
NGRP = SHARD // GTOK
MSUB = GTOK // 512  # 4 moving matmuls per stationary load
XTOK = 1024  # tokens per streamed x SBUF tile


def build_program(loop_reps: int = 0, bench_mode: bool = False):
    nc = bacc.Bacc("TRN2", debug=False, enable_asserts=True, num_devices=NCORES)
    io_kind = "Internal" if bench_mode else None
    # x, host layout [128, kd, tok]: input feature f = kd*128 + p
    x_d = nc.dram_tensor(
        "xt", [128, NKD * SHARD], dt.bfloat16, kind=io_kind or "ExternalInput"
    ).ap()
    # W, host layout [128, kd, o]: lhsT[k, o] = W[0].T[kd*128+p, o]
    w_d = nc.dram_tensor(
        "w0t", [128, NKD * HIDDEN], dt.bfloat16, kind="ExternalInput"
    ).ap()
    b_d = nc.dram_tensor("b0", [128, NOB], dt.float32, kind="ExternalInput").ap()
    outT_d = nc.dram_tensor(
        "outT", [HIDDEN, SHARD], dt.float16, kind=io_kind or "ExternalOutput"
    ).ap()
    done_d = (
        nc.dram_tensor("done", [1, NOB], dt.float32, kind="ExternalOutput").ap()
        if bench_mode
        else None
    )

    x_v = x_d.rearrange("p (kd n) -> p kd n", kd=NKD)
    w_v = w_d.rearrange("p (kd o) -> p kd o", kd=NKD)

    with tile.TileContext(nc) as tc:
        with ExitStack() as ctx:
            singles = ctx.enter_context(tc.tile_pool(name="singles", bufs=1))
            xpool = ctx.enter_context(tc.tile_pool(name="xpool", bufs=1))
            opool = ctx.enter_context(tc.tile_pool(name="opool", bufs=3))
            pspool = ctx.enter_context(tc.tile_pool(name="pspool", bufs=2, space="PSUM"))

            w_sb = singles.tile([128, NKD, HIDDEN], dt.bfloat16, name="w0t")
            nc.sync.dma_start(w_sb, w_v)
            bias = singles.tile([128, NOB], dt.float32, name="bias")
            nc.gpsimd.dma_start(bias, b_d)

            if bench_mode:
                zro = singles.tile([128, NKD * 256], dt.bfloat16, name="zro")
                nc.vector.memset(zro.bitcast(dt.uint16), 0)
                for zc in range(SHARD // 256):
                    s = slice(zc * NKD * 256, (zc + 1) * NKD * 256)
                    nc.sync.dma_start(x_d[:, s], zro)

            def body():
                xtiles = []
                for j in range(SHARD // XTOK):
                    ts = slice(j * XTOK, (j + 1) * XTOK)
                    xj = xpool.tile(
                        [128, NKD, XTOK], dt.bfloat16, name=f"x{j}", tag=f"x{j}"
                    )
                    nc.sync.dma_start(xj, x_v[:, :, ts])
                    xtiles.append(xj)

                for g in range(NGRP):
                    for ob in range(NOB):
                        ps = pspool.tile([128, GTOK], dt.float32, name="ps", tag="ps")
                        for kd in range(NKD):
                            lhsT = w_sb[:, kd, ob * 128 : (ob + 1) * 128]
                            for m in range(MSUB):
                                tok = g * GTOK + m * 512
                                j, off = divmod(tok, XTOK)
                                rhs = xtiles[j][:, kd, off : off + 512]
                                nc.tensor.matmul(
                                    ps[:, m * 512 : (m + 1) * 512],
                                    lhsT,
                                    rhs,
                                    start=(kd == 0),
                                    stop=(kd == NKD - 1),
                                )
                        osb = opool.tile([128, GTOK], dt.float16, name="osb", tag="osb")
                        if (g * NOB + ob) % 2 == 0:
                            nc.scalar.activation(
                                osb,
                                ps,
                                mybir.ActivationFunctionType.Identity,
                                bias=bias[:, ob : ob + 1],
                                scale=1.0,
                            )
                        else:
                            nc.vector.tensor_scalar(
                                osb,
                                ps,
                                bias[:, ob : ob + 1],
                                None,
                                mybir.AluOpType.add,
                            )
                        nc.sync.dma_start(
                            outT_d[ob * 128 : (ob + 1) * 128, g * GTOK : (g + 1) * GTOK],
                            osb,
                        )

            if loop_reps > 0:
                with tc.For_i(0, loop_reps, 1):
                    body()
            else:
                body()

            if done_d is not None:
                dsb = singles.tile([1, NOB], dt.float32, name="dsb")
                nc.vector.tensor_copy(dsb, bias[0:1, :])
                nc.sync.dma_start(done_d, dsb)

    nc.compile()
    return nc


_nc_cache: dict[tuple, object] = {}


def _get_nc(loop_reps: int = 0, bench_mode: bool = False):
    key = (loop_reps, bench_mode)
    if key not in _nc_cache:
        _nc_cache[key] = build_program(loop_reps, bench_mode)
    return _nc_cache[key]


def prepare_w_maps(W: np.ndarray, b: np.ndarray):
    Wt = np.ascontiguousarray(W[0].T).astype(BF16)  # [in, out]
    w0t = np.ascontiguousarray(
        Wt.reshape(NKD, 128, HIDDEN).transpose(1, 0, 2)
    ).reshape(128, -1)
    b0 = np.ascontiguousarray(b[0].reshape(NOB, 128).T).astype(np.float32)
    return {"w0t": w0t, "b0": b0}


def prepare_in_maps(x: np.ndarray, W: np.ndarray, b: np.ndarray):
    wmap = prepare_w_maps(W, b)
    in_maps = []
    for c in range(NCORES):
        x_c = np.asarray(x[c * SHARD : (c + 1) * SHARD], dtype=np.float32)
        xt = np.ascontiguousarray(
            x_c.T.astype(BF16).reshape(NKD, 128, SHARD).transpose(1, 0, 2)
        ).reshape(128, -1)
        in_maps.append({"xt": xt, **wmap})
    return in_maps


def kernel(x, routing_vectors, W, b):
    x = np.asarray(x, dtype=np.float32)
    W = np.asarray(W, dtype=np.float32)
    b = np.asarray(b, dtype=np.float32)
    nc = _get_nc(0)
    in_maps = prepare_in_maps(x, W, b)
    res = run_bass_kernel_spmd(nc, in_maps, core_ids=list(range(NCORES)))
    out = np.empty((BATCH, HIDDEN), dtype=np.float32)
    for c in range(NCORES):
        outT = np.asarray(res.results[c]["outT"])  # [feat, tok] fp16
        out[c * SHARD : (c + 1) * SHARD] = outT.astype(np.float32).T
    return out


# revision 4
# speedup vs baseline: 1.3764x; 1.0245x over previous
"""Trainium2 Bass kernel for nn_MixtureOfAdaptors (moe_routing).

out = x @ W[0].T + b[0] (routing collapses to expert 0), x [65536, 1024] fp32.

Mixed-precision K-split, data-parallel over 8 cores (8192 tokens each):
K-blocks 0..5 (768 of 1024 contraction dims) run in bf16 (1 cyc/col); the
last 2 K-blocks run as fp8-e4m3 DoubleRow matmuls (measured ~1.8-2x faster
per MAC on TRN2), single-rounded operands. Both accumulate into the same
fp32 PSUM group. Quantization noise from the fp8 quarter of K gives
rel err ~0.0164 vs the 2e-2 gate (full-fp8 would be 0.031). W is premultiplied
by 256 in BOTH streams (exact in bf16, keeps e4m3 out of subnormals) and the
drain descales by 1/256 while fusing the bias-add + fp16 convert in one
ACT/DVE pass. out.T layout [128 features, tokens] keeps bias per-partition.
PSUM: two ping-ponging 4-bank groups; token-major loop so next-iteration
x DMAs land behind compute.
"""

import sys

if "/opt/trn_rl_repo" not in sys.path:
    sys.path.insert(0, "/opt/trn_rl_repo")

from contextlib import ExitStack

import ml_dtypes
import numpy as np

import concourse.bass as bass
import concourse.tile as tile
from concourse import bacc, mybir
from concourse.bass_utils import run_bass_kernel_spmd

dt = mybir.dt
BF16 = ml_dtypes.bfloat16
F8 = ml_dtypes.float8_e4m3

BATCH = 65536
HIDDEN = 1024
NCORES = 8
SHARD = BATCH // NCORES  # 8192 tokens per core
KS = 768  # contraction split: k < KS in bf16, k >= KS in fp8 DoubleRow
NKD = KS // 128  # 6 bf16 contraction blocks
NOB = HIDDEN // 128  # 8 output-feature blocks
WSCALE = 256.0  # W premultiplier (both streams); descaled in the drain
GTOK = 2048  # tokens per PSUM group (4 banks of [128, 512] fp32)
NGRP = SHARD // GTOK
MSUB = GTOK // 512  # bf16 moving matmuls per stationary load
M8SUB = GTOK // 256  # fp8 DoubleRow matmuls per group
XTOK = 1024  # tokens per streamed x SBUF tile
NXT = SHARD // XTOK


def build_program(loop_reps: int = 0, bench_mode: bool = False):
    nc = bacc.Bacc("TRN2", debug=False, enable_asserts=True, num_devices=NCORES)
    io_kind = "Internal" if bench_mode else None
    # bf16 x, host layout [128, kd, tok]: feature f = kd*128 + p, f < KS
    xt_d = nc.dram_tensor(
        "xt", [128, NKD * SHARD], dt.bfloat16, kind=io_kind or "ExternalInput"
    ).ap()
    # fp8 x, host layout [128, i, tok]: feature f = KS + i*128 + p
    x8_d = nc.dram_tensor(
        "x8", [128, 2 * SHARD], dt.float8e4, kind=io_kind or "ExternalInput"
    ).ap()
    w_d = nc.dram_tensor(
        "w0t", [128, NKD * HIDDEN], dt.bfloat16, kind="ExternalInput"
    ).ap()
    w8_d = nc.dram_tensor(
        "w8", [128, 2 * HIDDEN], dt.float8e4, kind="ExternalInput"
    ).ap()
    b_d = nc.dram_tensor("b0", [128, NOB], dt.float32, kind="ExternalInput").ap()
    outT_d = nc.dram_tensor(
        "outT", [HIDDEN, SHARD], dt.float16, kind=io_kind or "ExternalOutput"
    ).ap()
    done_d = (
        nc.dram_tensor("done", [1, NOB], dt.float32, kind="ExternalOutput").ap()
        if bench_mode
        else None
    )

    xt_v = xt_d.rearrange("p (kd n) -> p kd n", kd=NKD)
    x8_v = x8_d.rearrange("p (i n) -> p i n", i=2)
    w_v = w_d.rearrange("p (kd o) -> p kd o", kd=NKD)
    w8_v = w8_d.rearrange("p (i o) -> p i o", i=2)

    with tile.TileContext(nc) as tc:
        with ExitStack() as ctx:
            singles = ctx.enter_context(tc.tile_pool(name="singles", bufs=1))
            xpool = ctx.enter_context(tc.tile_pool(name="xpool", bufs=1))
            opool = ctx.enter_context(tc.tile_pool(name="opool", bufs=3))
            pspool = ctx.enter_context(tc.tile_pool(name="pspool", bufs=2, space="PSUM"))

            w_sb = singles.tile([128, NKD, HIDDEN], dt.bfloat16, name="w0t")
            nc.sync.dma_start(w_sb, w_v)
            w8_sb = singles.tile([128, 2, HIDDEN], dt.float8e4, name="w8")
            nc.sync.dma_start(w8_sb, w8_v)
            bias = singles.tile([128, NOB], dt.float32, name="bias")
            nc.gpsimd.dma_start(bias, b_d)

            if bench_mode:
                zro = singles.tile([128, NKD * 256], dt.bfloat16, name="zro")
                nc.vector.memset(zro.bitcast(dt.uint16), 0)
                zro8 = singles.tile([128, 2 * 256], dt.float8e4, name="zro8")
                nc.vector.memset(zro8.bitcast(dt.uint8), 0)
                for zc in range(SHARD // 256):
                    nc.sync.dma_start(
                        xt_d[:, zc * NKD * 256 : (zc + 1) * NKD * 256], zro
                    )
                    nc.sync.dma_start(x8_d[:, zc * 2 * 256 : (zc + 1) * 2 * 256], zro8)

            def body():
                xbt, x8t = [], []
                for j in range(NXT):
                    ts = slice(j * XTOK, (j + 1) * XTOK)
                    xj = xpool.tile(
                        [128, NKD, XTOK], dt.bfloat16, name=f"xb{j}", tag=f"xb{j}"
                    )
                    nc.sync.dma_start(xj, xt_v[:, :, ts])
                    xbt.append(xj)
                    x8j = xpool.tile(
                        [128, 2, XTOK], dt.float8e4, name=f"x8{j}", tag=f"x8{j}"
                    )
                    nc.sync.dma_start(x8j, x8_v[:, :, ts])
                    x8t.append(x8j)

                for g in range(NGRP):
                    for ob in range(NOB):
                        ps = pspool.tile([128, GTOK], dt.float32, name="ps", tag="ps")
                        for kd in range(NKD):
                            lhsT = w_sb[:, kd, ob * 128 : (ob + 1) * 128]
                            for m in range(MSUB):
                                tok = g * GTOK + m * 512
                                j, off = divmod(tok, XTOK)
                                rhs = xbt[j][:, kd, off : off + 512]
                                nc.tensor.matmul(
                                    ps[:, m * 512 : (m + 1) * 512],
                                    lhsT,
                                    rhs,
                                    start=(kd == 0),
                                    stop=False,
                                )
                        # last quarter of K as fp8 DoubleRow (2 K-tiles of 128
                        # per matmul). Banks were started by the kd=0 bf16
                        # matmuls; each bank's second 256-half carries stop.
                        lhsT8 = w8_sb[:, :, ob * 128 : (ob + 1) * 128]
                        for m8 in range(M8SUB):
                            tok = g * GTOK + m8 * 256
                            j, off = divmod(tok, XTOK)
                            rhs8 = x8t[j][:, :, off : off + 256]
                            nc.tensor.matmul(
                                ps[:, m8 * 256 : (m8 + 1) * 256],
                                lhsT8,
                                rhs8,
                                start=False,
                                stop=(m8 % 2 == 1),
                                perf_mode=mybir.MatmulPerfMode.DoubleRow,
                            )
                        osb = opool.tile([128, GTOK], dt.float16, name="osb", tag="osb")
                        if (g * NOB + ob) % 2 == 0:
                            nc.scalar.activation(
                                osb,
                                ps,
                                mybir.ActivationFunctionType.Identity,
                                bias=bias[:, ob : ob + 1],
                                scale=1.0 / WSCALE,
                            )
                        else:
                            nc.vector.tensor_scalar(
                                osb,
                                ps,
                                1.0 / WSCALE,
                                bias[:, ob : ob + 1],
                                mybir.AluOpType.mult,
                                mybir.AluOpType.add,
                            )
                        nc.sync.dma_start(
                            outT_d[ob * 128 : (ob + 1) * 128, g * GTOK : (g + 1) * GTOK],
                            osb,
                        )

            if loop_reps > 0:
                with tc.For_i(0, loop_reps, 1):
                    body()
            else:
                body()

            if done_d is not None:
                dsb = singles.tile([1, NOB], dt.float32, name="dsb")
                nc.vector.tensor_copy(dsb, bias[0:1, :])
                nc.sync.dma_start(done_d, dsb)

    nc.compile()
    return nc


_nc_cache: dict[tuple, object] = {}


def _get_nc(loop_reps: int = 0, bench_mode: bool = False):
    key = (loop_reps, bench_mode)
    if key not in _nc_cache:
        _nc_cache[key] = build_program(loop_reps, bench_mode)
    return _nc_cache[key]


def prepare_w_maps(W: np.ndarray, b: np.ndarray):
    Wt = np.ascontiguousarray(W[0].T).astype(np.float32) * WSCALE  # [in, out]
    w0t = np.ascontiguousarray(
        Wt[:KS].astype(BF16).reshape(NKD, 128, HIDDEN).transpose(1, 0, 2)
    ).reshape(128, -1)
    w8 = np.ascontiguousarray(
        Wt[KS:].astype(F8).reshape(2, 128, HIDDEN).transpose(1, 0, 2)
    ).reshape(128, -1)
    b0 = np.ascontiguousarray(b[0].reshape(NOB, 128).T).astype(np.float32)
    return {"w0t": w0t, "w8": w8, "b0": b0}


def prepare_in_maps(x: np.ndarray, W: np.ndarray, b: np.ndarray):
    wmap = prepare_w_maps(W, b)
    in_maps = []
    for c in range(NCORES):
        xT = np.asarray(x[c * SHARD : (c + 1) * SHARD], dtype=np.float32).T
        xt = np.ascontiguousarray(
            xT[:KS].astype(BF16).reshape(NKD, 128, SHARD).transpose(1, 0, 2)
        ).reshape(128, -1)
        x8 = np.ascontiguousarray(
            xT[KS:].astype(F8).reshape(2, 128, SHARD).transpose(1, 0, 2)
        ).reshape(128, -1)
        in_maps.append({"xt": xt, "x8": x8, **wmap})
    return in_maps


def kernel(x, routing_vectors, W, b):
    x = np.asarray(x, dtype=np.float32)
    W = np.asarray(W, dtype=np.float32)
    b = np.asarray(b, dtype=np.float32)
    nc = _get_nc(0)
    in_maps = prepare_in_maps(x, W, b)
    res = run_bass_kernel_spmd(nc, in_maps, core_ids=list(range(NCORES)))
    out = np.empty((BATCH, HIDDEN), dtype=np.float32)
    for c in range(NCORES):
        outT = np.asarray(res.results[c]["outT"])  # [feat, tok] fp16
        out[c * SHARD : (c + 1) * SHARD] = outT.astype(np.float32).T
    return out
